# revision 6
# baseline (speedup 1.0000x reference)
"""Trainium2 Bass kernel for entmax-1.5 via n-section (nn_EntmaxNsect).

Full input: X [4096, 32000] f32 -> output entmax weights, same shape.
Data-parallel over 8 NeuronCores (512 rows each), row-blocks of 128 per core.

fp16 datapath (validated offline vs the jax reference; rel err ~2.2e-3):
  host casts X to fp16 (read traffic halves), output written fp16 (write
  traffic halves) and cast back to f32 on the host. alpha=1.5 ->
  p = relu(0.5x - tau)^2 / Z. Work in th2 = 2*tau space so x is used raw.
  tau ladder: chunkmax-128 Newton x4, chunkmax-8 Newton x2, then one
  full-data Newton round (z = relu(x - th2) kept in SBUF fp16).
  Output p = Square(sc*z + bi) on ACT with sc = 0.5*sv, bi = -sc*dd2:
  absorbing the final tau shift dd2 into the activation bias instead of
  re-thresholding x; error <= (sc*dd2)^2 ~ 2.5e-5 absolute.
  Z = 1 + Nh*(dd2/2)^2 to second order; 1/sqrt(Z) ~= 1 - (Z-1)/2.

Raw-bass, explicit engines + semaphores. d() = engine drain for same-engine
pipeline hazards (consecutive engine ops overlap in the HW pipeline).
Engine budget per 128-row block: DVE ~27k cyc (maxtree fp16 4x, ladders,
z-pass tensor_scalar 4x), ACT 2x32000 (eval Square+accum, out Square),
DMA 16MB fp16 in+out.
"""

import contextlib
import numpy as np

P = 128
D = 32000
W8 = 8
N8 = D // W8          # 4000
W16 = 16
N128 = N8 // W16      # 250
CW = 4000             # chunk width (load / eval / out)
NCH = D // CW         # 8
COARSE_ITERS = 4
FINE_ITERS = 2
NO2 = 3               # out-buffer rotation depth


def build_entmax_kernel(nc, n_rows, reps=1):
    import concourse.mybir as mybir
    f16 = mybir.dt.float16
    x = nc.dram_tensor("X", [n_rows, D], f16, kind="ExternalInput")
    out = nc.dram_tensor("OUT", [n_rows, D], f16, kind="ExternalOutput")
    return build_entmax_kernel_aps(nc, x[:, :], out[:, :], n_rows, reps)


def build_entmax_kernel_aps(nc, x, out, n_rows, reps=1):
    import concourse.mybir as mybir

    f32 = mybir.dt.float32
    f16 = mybir.dt.float16
    AX = mybir.AxisListType.X
    OP = mybir.AluOpType
    AF = mybir.ActivationFunctionType

    nblk_real = n_rows // P
    nblk = nblk_real * reps
    assert n_rows % P == 0

    def rowbase(b):
        return (b % nblk_real) * P

    ctx = contextlib.ExitStack()
    with ctx:
        _n = [0]

        def sb(shape, dt=f32):
            _n[0] += 1
            return ctx.enter_context(nc.sbuf_tensor(f"t{_n[0]}", shape, dt))

        xb = sb([P, D], f16)          # raw input block
        zb = sb([P, D], f16)          # z = relu(x - th2), retained full block
        m8 = sb([P, N8], f16)
        m128 = sb([P, N128], f16)
        lad = sb([P, N8], f16)        # ladder scratch + Nh mask scratch
        lad2 = sb([P, N8], f16)       # ladder squares (avoids in-place WAR)
        trash = sb([P, CW], f16)      # eval-square mandatory output
        o2 = [sb([P, CW], f16) for _ in range(NO2)]

        def sm2(n=1):
            return [sb([P, n]) for _ in range(2)]
        Bp, Ap = sm2(NCH), sm2(NCH)
        sc_t, bi_t = sm2(), sm2()
        mxx, th2, lo02, rr2 = (sb([P, 1]) for _ in range(4))
        lnum, den, rec = (sb([P, 1]) for _ in range(3))
        Nh, dd2, dsq, zt, sv = (sb([P, 1]) for _ in range(5))
        Bx, Axm = sb([P, 1]), sb([P, 1])

        s_load = ctx.enter_context(nc.semaphore("s_load"))
        s_z = ctx.enter_context(nc.semaphore("s_z"))
        s_A = ctx.enter_context(nc.semaphore("s_A"))
        s_th = ctx.enter_context(nc.semaphore("s_th"))
        s_out = ctx.enter_context(nc.semaphore("s_out"))
        s_od = [ctx.enter_context(nc.semaphore(f"s_od{j}"))
                for j in range(NO2)]

        block = ctx.enter_context(nc.Block())

        @block.sync
        def _(sp):
            for b in range(nblk):
                r0 = rowbase(b)
                for c in range(NCH):
                    # x chunk slot is free once block b-1's z chunk c is
                    # computed (z-pass is the last reader of xb)
                    if b > 0:
                        sp.wait_ge(s_z, NCH * (b - 1) + c + 1)
                    sp.dma_start(
                        xb[:, c * CW:(c + 1) * CW],
                        x[r0:r0 + P, c * CW:(c + 1) * CW]
                    ).then_inc(s_load, 16)

        @block.vector
        def _(dve):
            d = dve.drain
            for b in range(nblk):
                i = b % 2
                npc = CW // W8
                # hierarchical max tree, per chunk as loads land
                for c in range(NCH):
                    dve.wait_ge(s_load, 16 * (NCH * b + c + 1))
                    dve.tensor_reduce(
                        m8[:, c * npc:(c + 1) * npc],
                        xb[:, c * CW:(c + 1) * CW].rearrange(
                            "p (c w) -> p c w", w=W8),
                        axis=AX, op=OP.max)
                d()
                dve.tensor_reduce(
                    m128[:], m8[:].rearrange("p (c w) -> p c w", w=W16),
                    axis=AX, op=OP.max)
                d()
                dve.tensor_reduce(mxx[:], m128[:], axis=AX, op=OP.max)
                d()
                dve.tensor_scalar_add(th2[:], mxx[:], -1.0)
                dve.tensor_scalar_add(lo02[:], mxx[:], -2.0)
                d()

                # Newton ladder iteration on proxy data (th2-space):
                # th2 += (0.5*sum(z^2) - 2) / sum(z),  z = relu(vals - th2)
                # NOTE tensor_scalar+accum_out is a reduce variant (op1 is
                # the reduce op), so sums come from explicit tensor_reduce /
                # scalar_tensor_tensor instead.
                def ladder(vals, width, clamp):
                    dve.tensor_scalar(
                        lad[:, :width], vals[:, :width], th2[:], th2[:],
                        op0=OP.max, op1=OP.subtract)
                    d()
                    dve.tensor_reduce(Bx[:], lad[:, :width], axis=AX,
                                      op=OP.add)
                    dve.scalar_tensor_tensor(
                        lad2[:, :width], lad[:, :width], 1.0, lad[:, :width],
                        op0=OP.mult, op1=OP.mult, accum_out=Axm[:])
                    d()
                    dve.tensor_scalar(lnum[:], Axm[:], 0.5, -2.0,
                                      op0=OP.mult, op1=OP.add)
                    dve.tensor_scalar_max(den[:], Bx[:], 1e-20)
                    d()
                    dve.reciprocal(rec[:], den[:])
                    d()
                    dve.scalar_tensor_tensor(th2[:], lnum[:], rec[:], th2[:],
                                             op0=OP.mult, op1=OP.add)
                    d()
                    if clamp:
                        dve.scalar_tensor_tensor(th2[:], th2[:], lo02[:],
                                                 mxx[:], op0=OP.max,
                                                 op1=OP.min)
                        d()

                for it in range(COARSE_ITERS):
                    ladder(m128, N128, clamp=(it == COARSE_ITERS - 1))
                for it in range(FINE_ITERS):
                    ladder(m8, N8, clamp=True)

                # support count from m8 (second-order normalization term)
                dve.tensor_scalar(lad[:], m8[:], th2[:], 1.0,
                                  op0=OP.is_gt, op1=OP.mult)
                d()
                dve.tensor_reduce(Nh[:], lad[:], axis=AX, op=OP.add)

                # full-data z pass: z = relu(x - th2)
                for c in range(NCH):
                    if b > 0:
                        dve.wait_ge(s_out, NCH * (b - 1) + c + 1)
                    dve.tensor_scalar(
                        zb[:, c * CW:(c + 1) * CW],
                        xb[:, c * CW:(c + 1) * CW],
                        th2[:], th2[:], op0=OP.max, op1=OP.subtract
                    ).then_inc(s_z, 1)
                d()
                # B per chunk (z writes drained above)
                for c in range(NCH):
                    dve.tensor_reduce(Bp[i][:, c:c + 1],
                                      zb[:, c * CW:(c + 1) * CW],
                                      axis=AX, op=OP.add)
                d()

                # final Newton with exact full-data sums (A from ACT)
                dve.wait_ge(s_A, NCH * (b + 1))
                dve.tensor_reduce(Bx[:], Bp[i][:], axis=AX, op=OP.add)
                dve.tensor_reduce(Axm[:], Ap[i][:], axis=AX, op=OP.add)
                d()
                dve.tensor_scalar(lnum[:], Axm[:], 0.5, -2.0,
                                  op0=OP.mult, op1=OP.add)
                dve.tensor_scalar_max(den[:], Bx[:], 1e-20)
                d()
                dve.reciprocal(rec[:], den[:])
                d()
                dve.scalar_tensor_tensor(rr2[:], lnum[:], rec[:], th2[:],
                                         op0=OP.mult, op1=OP.add)
                d()
                dve.scalar_tensor_tensor(rr2[:], rr2[:], lo02[:], mxx[:],
                                         op0=OP.max, op1=OP.min)
                d()
                dve.tensor_sub(dd2[:], rr2[:], th2[:])
                d()
                dve.tensor_mul(dsq[:], dd2[:], dd2[:])
                d()
                dve.tensor_mul(zt[:], dsq[:], Nh[:])
                d()
                # sv = 1 - Nh*(dd2/2)^2/2 = 1 - Nh*dd2^2/8
                dve.tensor_scalar(sv[:], zt[:], -0.125, 1.0,
                                  op0=OP.mult, op1=OP.add)
                d()
                dve.tensor_scalar_mul(sc_t[i][:], sv[:], 0.5)
                d()
                dve.scalar_tensor_tensor(bi_t[i][:], sc_t[i][:], -1.0,
                                         dd2[:], op0=OP.mult, op1=OP.mult)
                d()
                dve.nop().then_inc(s_th, 1)

        @block.scalar
        def _(act):
            for b in range(nblk):
                i = b % 2
                # exact sum of squares for normalization (accum to Ap)
                for c in range(NCH):
                    act.wait_ge(s_z, NCH * b + c + 1)
                    act.activation(trash[:], zb[:, c * CW:(c + 1) * CW],
                                   AF.Square,
                                   accum_out=Ap[i][:, c:c + 1]
                                   ).then_inc(s_A, 1)
                act.wait_ge(s_th, b + 1)
                r0 = rowbase(b)
                for c in range(NCH):
                    oc = NCH * b + c
                    if oc >= NO2:
                        act.wait_ge(s_od[oc % NO2], 16 * (oc // NO2))
                    ob = o2[oc % NO2]
                    # p = Square(sc*z + bi) = (sc*(relu(x-th2) - dd2))^2
                    act.activation(ob[:], zb[:, c * CW:(c + 1) * CW],
                                   AF.Square, bias=bi_t[i][:],
                                   scale=sc_t[i][:]).then_inc(s_out, 1)
                    act.drain()
                    col = c * CW
                    act.dma_start(
                        out[r0:r0 + P, col:col + CW], ob[:]
                    ).then_inc(s_od[oc % NO2], 16)

    return nc


_CACHE = {}


def _get_nc(n_rows, reps=1):
    key = (n_rows, reps)
    if key in _CACHE:
        return _CACHE[key]
    import concourse.bass as bass

    nc = bass.Bass("TRN2")
    build_entmax_kernel(nc, n_rows, reps)
    nc.finalize()
    _CACHE[key] = nc
    return nc


N_CORES = 8
CHUNK_PER_CORE = 128          # rows per core per pipelined chunk
CHUNK = N_CORES * CHUNK_PER_CORE


_STATE = {}


def _build_exec():
    """Compile the per-chunk SPMD callable once; cache in module state.

    Bypasses run_bass_kernel_spmd (which re-jits a fresh closure and
    round-trips zero-filled output buffers through the host on every call).
    The kernel writes every OUT element, so no zero operands are passed —
    the custom call's result buffer is left to PJRT.
    """
    import jax
    from jax.sharding import Mesh, NamedSharding, PartitionSpec
    from jax.experimental.shard_map import shard_map
    import concourse.mybir as mybir
    from concourse import bass2jax

    nc = _get_nc(CHUNK_PER_CORE)
    bass2jax.install_neuronx_cc_hook()
    partition_name = (nc.partition_id_tensor.name
                      if nc.partition_id_tensor else None)

    in_names, out_names, out_avals = [], [], []
    for alloc in nc.m.functions[0].allocations:
        if not isinstance(alloc, mybir.MemoryLocationSet):
            continue
        name = alloc.memorylocations[0].name
        if alloc.kind == "ExternalInput":
            if name != partition_name:
                in_names.append(name)
        elif alloc.kind == "ExternalOutput":
            out_names.append(name)
            out_avals.append(jax.core.ShapedArray(
                tuple(alloc.tensor_shape), mybir.dt.np(alloc.dtype)))
    all_in = list(in_names)
    if partition_name is not None:
        all_in.append(partition_name)

    def _body(*args):
        operands = list(args)
        if partition_name is not None:
            operands.append(bass2jax.partition_id_tensor())
        return tuple(bass2jax._bass_exec_p.bind(
            *operands,
            out_avals=tuple(out_avals),
            in_names=tuple(all_in),
            out_names=tuple(out_names),
            lowering_input_output_aliases=(),
            sim_require_finite=True,
            sim_require_nnan=True,
            nc=nc,
        ))

    devices = jax.devices()[:N_CORES]
    mesh = Mesh(np.asarray(devices), ("core",))
    sh = NamedSharding(mesh, PartitionSpec("core"))
    fn = jax.jit(
        shard_map(_body, mesh=mesh,
                  in_specs=(PartitionSpec("core"),),
                  out_specs=(PartitionSpec("core"),),
                  check_rep=False),
    )
    compiled = fn.lower(
        jax.ShapeDtypeStruct((CHUNK, D), np.float16, sharding=sh)).compile()
    _STATE.update(fn=compiled, sh=sh)
    return _STATE


def _to_f16(X):
    """Parallel f32 -> fp16 cast (astype releases the GIL on big blocks)."""
    import concurrent.futures as cf
    out = np.empty(X.shape, np.float16)
    n = X.shape[0]
    step = max(1, (n + 7) // 8)

    def conv(k):
        out[k:k + step] = X[k:k + step]

    with cf.ThreadPoolExecutor(8) as ex:
        list(ex.map(conv, range(0, n, step)))
    return out


def kernel(X: np.ndarray) -> np.ndarray:
    import jax

    st = _STATE or _build_exec()
    fn, sh = st["fn"], st["sh"]
    X = np.ascontiguousarray(X, dtype=np.float32)
    Xh = _to_f16(X)
    rows = X.shape[0]
    n_chunks = rows // CHUNK

    # Pipeline: enqueue all uploads + execs asynchronously, then drain
    # outputs in order so D2H of chunk i overlaps H2D/exec of later chunks.
    outs = []
    for i in range(n_chunks):
        xd = jax.device_put(Xh[i * CHUNK:(i + 1) * CHUNK], sh)
        (o,) = fn(xd)
        o.copy_to_host_async()
        outs.append(o)

    # Place each device shard straight into the result (converting fp16 ->
    # f32 during the assignment): np.asarray on the sharded array would
    # first assemble a chunk-sized intermediate before our slice-assign.
    res = np.empty((rows, D), np.float32)
    for i, o in enumerate(outs):
        base = i * CHUNK
        for s in o.addressable_shards:
            r = s.index[0]
            lo = 0 if r.start is None else r.start
            hi = CHUNK if r.stop is None else r.stop
            res[base + lo:base + hi] = np.asarray(s.data)
    return res
